# revision 1
# baseline (speedup 1.0000x reference)
"""BiMamba layer Trainium2 kernel (8 NeuronCores, SPMD).

Sharding: 4 batch-groups x 2 d_inner-halves. Core (g, h) handles the 3
(b*f) scan units of batch g for d_inner channels [96h, 96h+96), both scan
directions. Each core emits a partial out-projection; the host sums the
two halves per batch and adds out_proj_b.

Per-core pipeline (per unit):
  conv+in_proj fused (PE, f32r, host-precomputed shifted matrices M_k)
  -> silu (ACT) -> x_proj / dt_proj (PE) -> softplus via Ln(Exp(p)+1) (ACT)
  -> delta replication to (d,n) rows (PE selection matmul -> PSUM)
  -> dA = Exp(A * delta) with per-partition A scale (ACT, bf16)
  -> dBu = du_rep * B_rep (GPSIMD TT bf16; du_rep via DMA broadcast)
  -> h = tensor_tensor_scan (DVE, fwd + time-reversed-AP bwd)
  -> hC (DVE TT bf16) -> n-reduction (PE selection matmul, + 2*u*D term)
  -> out_proj partial (PE f32r) -> HBM
"""
from contextlib import ExitStack

import numpy as np

import concourse.bass as bass
import concourse.tile as tile
from concourse import bacc, mybir
from concourse.bass_utils import run_bass_kernel_spmd

F32 = mybir.dt.float32
FR = mybir.dt.float32r
BF = mybir.dt.bfloat16
AF = mybir.ActivationFunctionType
OP = mybir.AluOpType

B, SEQ, DIM = 4, 6144, 384
L = 2048                  # per-unit sequence length
NU = 3                    # units per core
DIN, DH, NST, DTR = 192, 96, 16, 24
NK = 12                   # (DH*NST)//128 row-tiles
LC = 512                  # psum column chunk
NLC = L // LC
NCORES = 8

_NC_CACHE = {}


def _build(ab_same: bool, debug: bool = False):
    nc = bacc.Bacc("TRN2", target_bir_lowering=False, debug=False)

    def din(name, shape, dt=F32):
        return nc.dram_tensor(name, list(shape), dt, kind="ExternalInput").ap()

    xtp_d = din("xtp", (NU, 3, 128, L + 2))
    wm_d = din("wm", (9, 128, DIN))
    wmu_d = din("wmu", (9, 128, DH))
    efix_d = din("efix", (1, 2, DIN))
    efixu_d = din("efixu", (1, 2, DH))
    one_d = din("one", (1, 1))
    bsil_d = din("bsil", (128, 2))
    ubias_d = din("ubias", (DH, 1))
    wxp_d = din("wxp", (DIN, 56))
    wxpbc_d = din("wxpbc", (DIN, 256))
    wdt_d = din("wdt", (DTR, DH))
    bsp_d = din("bsp", (DH, 1))
    acol_d = din("acol", (NK * 128,))
    abcol_d = din("abcol", (NK * 128,))
    seli_d = din("seli", (DH, NK * 128))
    seln_d = din("seln", (4, 128, 32), BF)
    ddiag_d = din("ddiag", (DH, DH))
    wouty_d = din("wouty", (DH, DIM))
    woutz_d = din("woutz", (DH, DIM))
    out_d = nc.dram_tensor("out", [NU, L, DIM], F32, kind="ExternalOutput").ap()
    dbg = {}
    if debug:
        for name, shape, dt_ in [
                ("dbg_xc0", (128, L), F32), ("dbg_u", (DH, L), F32),
                ("dbg_dt", (DTR, L), F32), ("dbg_bc", (32, L), BF),
                ("dbg_delta", (DH, L), F32), ("dbg_du", (DH, L), BF),
                ("dbg_brep", (128, L), BF), ("dbg_daf", (128, L), BF),
                ("dbg_dbu", (128, L), BF), ("dbg_hf", (128, L), BF),
                ("dbg_hcb", (128, L), BF), ("dbg_y", (DH, L), F32)]:
            dbg[name] = nc.dram_tensor(name, list(shape), dt_,
                                       kind="ExternalOutput").ap()

    with tile.TileContext(nc) as tc, ExitStack() as ctx:
        cp = ctx.enter_context(tc.tile_pool(name="consts", bufs=1))
        px = ctx.enter_context(tc.tile_pool(name="px", bufs=1))
        pxc = ctx.enter_context(tc.tile_pool(name="pxc", bufs=1))
        pu = ctx.enter_context(tc.tile_pool(name="pu", bufs=2))
        psm = ctx.enter_context(tc.tile_pool(name="psm", bufs=1))
        pbig = ctx.enter_context(tc.tile_pool(name="pbig", bufs=2))
        pout = ctx.enter_context(tc.tile_pool(name="pout", bufs=2))
        ppa = ctx.enter_context(tc.tile_pool(name="ppa", bufs=2, space="PSUM"))
        ppd = ctx.enter_context(tc.tile_pool(name="ppd", bufs=2, space="PSUM"))
        ppy = ctx.enter_context(tc.tile_pool(name="ppy", bufs=4, space="PSUM"))

        # ---- constants ----
        wm_sb = cp.tile([128, 9, DIN], FR)
        nc.sync.dma_start(wm_sb[:], wm_d.transpose([1, 0, 2]).bitcast(FR))
        wmu_sb = cp.tile([128, 9, DH], FR)
        nc.sync.dma_start(wmu_sb[:], wmu_d.transpose([1, 0, 2]).bitcast(FR))
        efix_sb = cp.tile([1, 2, DIN], F32)
        nc.sync.dma_start(efix_sb[:], efix_d)
        efixu_sb = cp.tile([1, 2, DH], F32)
        nc.sync.dma_start(efixu_sb[:], efixu_d)
        one_sb = cp.tile([1, 1], F32)
        nc.sync.dma_start(one_sb[:], one_d)
        bsil_sb = cp.tile([128, 2], F32)
        nc.sync.dma_start(bsil_sb[:], bsil_d)
        ubias_sb = cp.tile([DH, 1], F32)
        nc.sync.dma_start(ubias_sb[:], ubias_d)
        wxp_sb = cp.tile([128, 56], FR)
        nc.sync.dma_start(wxp_sb[:], wxp_d[0:128, :].bitcast(FR))
        wxp2_sb = cp.tile([64, 56], FR)
        nc.sync.dma_start(wxp2_sb[:], wxp_d[128:192, :].bitcast(FR))
        wxpbc_sb = cp.tile([128, 256], FR)
        nc.sync.dma_start(wxpbc_sb[:], wxpbc_d[0:128, :].bitcast(FR))
        wxpbc2_sb = cp.tile([64, 256], FR)
        nc.sync.dma_start(wxpbc2_sb[:], wxpbc_d[128:192, :].bitcast(FR))
        wdt_sb = cp.tile([DTR, DH], FR)
        nc.sync.dma_start(wdt_sb[:], wdt_d.bitcast(FR))
        bsp_sb = cp.tile([DH, 1], F32)
        nc.sync.dma_start(bsp_sb[:], bsp_d)
        acol_sb = cp.tile([128, NK], F32)
        nc.sync.dma_start(acol_sb[:], acol_d.rearrange("(k p) -> p k", p=128))
        abcol_sb = cp.tile([128, NK], F32)
        nc.sync.dma_start(abcol_sb[:], abcol_d.rearrange("(k p) -> p k", p=128))
        seli_sb = cp.tile([DH, NK * 128], FR)
        nc.sync.dma_start(seli_sb[:], seli_d.bitcast(FR))
        seln_sb = cp.tile([128, 4, 32], BF)
        nc.sync.dma_start(seln_sb[:], seln_d.transpose([1, 0, 2]))
        ddiag_sb = cp.tile([DH, DH], FR)
        nc.sync.dma_start(ddiag_sb[:], ddiag_d.bitcast(FR))
        wouty_sb = cp.tile([DH, DIM], FR)
        nc.sync.dma_start(wouty_sb[:], wouty_d.bitcast(FR))
        woutz_sb = cp.tile([DH, DIM], FR)
        nc.sync.dma_start(woutz_sb[:], woutz_d.bitcast(FR))

        for u in range(NU):
            xt = px.tile([128, 3, L + 2], FR)
            nc.sync.dma_start(xt[:], xtp_d[u].transpose([1, 0, 2]).bitcast(FR))

            # ---- fused conv + in_proj ----
            xc0 = pxc.tile([128, L], FR)
            xc1 = pxc.tile([64, L], FR)
            u_sb = pu.tile([DH, L], FR)
            # three output groups: xc0 (ch 0:128), xc1 (ch 128:192), u (d-half)
            groups = [
                (wm_sb, 0, 128, xc0, bsil_sb[0:128, 0:1], efix_sb),
                (wm_sb, 128, 64, xc1, bsil_sb[0:64, 1:2], efix_sb),
                (wmu_sb, 0, DH, u_sb, ubias_sb[:], efixu_sb),
            ]
            for lc in range(NLC):
                for wsrc, c0, cw, dst, bias_ap, efx in groups:
                    ps = ppa.tile([128, LC], F32, tag="ppa", name="ps_conv")
                    mms = []
                    for s in range(3):
                        for kt in range(3):
                            mms.append((ps[0:cw, :],
                                        wsrc[:, s * 3 + kt, c0:c0 + cw],
                                        xt[:, kt, s + lc * LC:s + lc * LC + LC]))
                    if lc == 0:
                        mms.append((ps[0:cw, 0:1],
                                    efx[0:1, 0, c0:c0 + cw], one_sb[:]))
                    if lc == NLC - 1:
                        mms.append((ps[0:cw, LC - 1:LC],
                                    efx[0:1, 1, c0:c0 + cw], one_sb[:]))
                    for i, (o, lh, rh) in enumerate(mms):
                        nc.tensor.matmul(o, lh, rh, start=(i == 0),
                                         stop=(i == len(mms) - 1))
                    nc.scalar.activation(dst[:, lc * LC:(lc + 1) * LC],
                                         ps[0:cw, :], AF.Silu, bias=bias_ap)

            # ---- x_proj -> dt, and B/C already replicated to 128 rows ----
            dt_sb = psm.tile([DTR, L], FR)
            brep = psm.tile([128, L], BF)
            crep = psm.tile([128, L], BF)
            crep_rev = psm.tile([128, L], BF)
            xc0r = xc0[:, ::-1]
            xc1r = xc1[:, ::-1]
            for lc in range(NLC):
                sl = slice(lc * LC, (lc + 1) * LC)
                pdt = ppa.tile([128, LC], F32, tag="ppa", name="ps_dt")
                nc.tensor.matmul(pdt[0:DTR, :], wxp_sb[:, 0:DTR], xc0[:, sl],
                                 start=True, stop=False)
                nc.tensor.matmul(pdt[0:DTR, :], wxp2_sb[:, 0:DTR], xc1[:, sl],
                                 start=False, stop=True)
                nc.scalar.activation(dt_sb[:, sl], pdt[0:DTR, :], AF.Copy)
                pbr = ppa.tile([128, LC], F32, tag="ppa", name="ps_br")
                nc.tensor.matmul(pbr[:], wxpbc_sb[:, 0:128], xc0[:, sl],
                                 start=True, stop=False)
                nc.tensor.matmul(pbr[:], wxpbc2_sb[:, 0:128], xc1[:, sl],
                                 start=False, stop=True)
                nc.scalar.activation(brep[:, sl], pbr[:], AF.Copy)
                pcr = ppa.tile([128, LC], F32, tag="ppa", name="ps_cr")
                nc.tensor.matmul(pcr[:], wxpbc_sb[:, 128:256], xc0[:, sl],
                                 start=True, stop=False)
                nc.tensor.matmul(pcr[:], wxpbc2_sb[:, 128:256], xc1[:, sl],
                                 start=False, stop=True)
                nc.scalar.activation(crep[:, sl], pcr[:], AF.Copy)
                pcrr = ppa.tile([128, LC], F32, tag="ppa", name="ps_crr")
                nc.tensor.matmul(pcrr[:], wxpbc_sb[:, 128:256], xc0r[:, sl],
                                 start=True, stop=False)
                nc.tensor.matmul(pcrr[:], wxpbc2_sb[:, 128:256], xc1r[:, sl],
                                 start=False, stop=True)
                nc.scalar.activation(crep_rev[:, sl], pcrr[:], AF.Copy)

            if debug and u == 0:
                nc.sync.dma_start(dbg["dbg_xc0"], xc0[:].bitcast(F32))
                nc.sync.dma_start(dbg["dbg_u"], u_sb[:].bitcast(F32))
                nc.sync.dma_start(dbg["dbg_dt"], dt_sb[:].bitcast(F32))

            # ---- dt_proj + softplus ----
            delta_sb = psm.tile([DH, L], FR, bufs=2)
            esp = psm.tile([DH, L], F32)
            for lc in range(NLC):
                sl = slice(lc * LC, (lc + 1) * LC)
                pdp = ppa.tile([128, LC], F32, tag="ppa", name="ps_dp")
                nc.tensor.matmul(pdp[0:DH, :], wdt_sb[:], dt_sb[:, sl],
                                 start=True, stop=True)
                nc.scalar.activation(esp[:, sl], pdp[0:DH, :], AF.Exp,
                                     bias=bsp_sb[:])
            nc.scalar.activation(delta_sb[:], esp[:], AF.Ln, bias=1.0)

            # ---- du = delta * u (bf16) ----
            du_sb = psm.tile([DH, L], BF, bufs=2)
            nc.vector.tensor_tensor(du_sb[:], delta_sb[:], u_sb[:], OP.mult)


            if debug and u == 0:
                nc.sync.dma_start(dbg["dbg_delta"], delta_sb[:].bitcast(F32))
                nc.sync.dma_start(dbg["dbg_du"], du_sb[:])
            y_sb = psm.tile([DH, L], FR)

            # ---- main scan loop over row-tile groups ----
            for kg in range(3):
                pys = [ppy.tile([32, LC], F32, tag="pys", name="pys")
                       for _ in range(NLC)]
                for kk in range(4):
                    k = 4 * kg + kk
                    durep = pbig.tile([128, L], BF)
                    nc.gpsimd.dma_start(
                        durep[:],
                        du_sb[8 * k:8 * k + 8, :].unsqueeze(1)
                        .broadcast_to([8, 16, L]))
                    daf = pbig.tile([128, L], BF)
                    dab = None if ab_same else pbig.tile([128, L], BF)
                    for lc in range(NLC):
                        pd = ppd.tile([128, LC], F32)
                        nc.tensor.matmul(pd[:], seli_sb[:, 128 * k:128 * (k + 1)],
                                         delta_sb[:, lc * LC:(lc + 1) * LC],
                                         start=True, stop=True)
                        nc.scalar.activation(daf[:, lc * LC:(lc + 1) * LC],
                                             pd[:], AF.Exp,
                                             scale=acol_sb[:, k:k + 1])
                        if not ab_same:
                            nc.scalar.activation(dab[:, lc * LC:(lc + 1) * LC],
                                                 pd[:], AF.Exp,
                                                 scale=abcol_sb[:, k:k + 1])
                    dbu = pbig.tile([128, L], BF)
                    nc.gpsimd.tensor_tensor(dbu[:], durep[:], brep[:], OP.mult)
                    hf = pbig.tile([128, L], BF)
                    nc.vector.tensor_tensor_scan(hf[:], daf[:], dbu[:], 0.0,
                                                 OP.mult, OP.add)
                    hb = pbig.tile([128, L], BF)
                    dab_src = daf if ab_same else dab
                    nc.vector.tensor_tensor_scan(hb[:], dab_src[:, ::-1],
                                                 dbu[:, ::-1], 0.0,
                                                 OP.mult, OP.add)
                    hcf = pbig.tile([128, L], BF)
                    nc.vector.tensor_tensor(hcf[:], hf[:], crep[:], OP.mult)
                    hcb = pbig.tile([128, L], BF)
                    nc.vector.tensor_tensor(hcb[:], hb[:], crep_rev[:],
                                            OP.mult)
                    if debug and u == 0 and k == 0:
                        nc.sync.dma_start(dbg["dbg_brep"], brep[:])
                        nc.sync.dma_start(dbg["dbg_daf"], daf[:])
                        nc.sync.dma_start(dbg["dbg_dbu"], dbu[:])
                        nc.sync.dma_start(dbg["dbg_hf"], hf[:])
                        nc.sync.dma_start(dbg["dbg_hcb"], hcb[:])
                    for lc in range(NLC):
                        sl = slice(lc * LC, (lc + 1) * LC)
                        nc.tensor.matmul(pys[lc][:], seln_sb[:, kk, :],
                                         hcf[:, sl], start=(kk == 0), stop=False)
                        nc.tensor.matmul(
                            pys[lc][:], seln_sb[:, kk, :],
                            hcb[:, ::-1][:, sl], start=False, stop=False)
                for lc in range(NLC):
                    sl = slice(lc * LC, (lc + 1) * LC)
                    nc.tensor.matmul(pys[lc][:], ddiag_sb[:, 32 * kg:32 * kg + 32],
                                     u_sb[:, sl], start=False, stop=True)
                    nc.scalar.activation(y_sb[32 * kg:32 * kg + 32, sl],
                                         pys[lc][:], AF.Copy)

            if debug and u == 0:
                nc.sync.dma_start(dbg["dbg_y"], y_sb[:].bitcast(F32))

            # ---- out_proj partial ----
            for tq in range(L // 256):
                osb = pout.tile([128, 2, DIM], F32)
                for j in range(2):
                    t8 = tq * 2 + j
                    sl = slice(t8 * 128, (t8 + 1) * 128)
                    po = ppa.tile([128, LC], F32, tag="ppa", name="ps_o")
                    nc.tensor.matmul(po[:, 0:DIM], y_sb[:, sl], wouty_sb[:],
                                     start=True, stop=False)
                    nc.tensor.matmul(po[:, 0:DIM], u_sb[:, sl], woutz_sb[:],
                                     start=False, stop=True)
                    nc.scalar.activation(osb[:, j, :], po[:, 0:DIM], AF.Copy)
                nc.sync.dma_start(
                    out_d[u, tq * 256:(tq + 1) * 256, :]
                    .rearrange("(j p) c -> p j c", p=128),
                    osb[:])

    nc.compile()
    return nc


def _get_nc(ab_same: bool):
    if ab_same not in _NC_CACHE:
        _NC_CACHE[ab_same] = _build(ab_same)
    return _NC_CACHE[ab_same]


def _prep_weights(h, in_proj_w, in_proj_b, conv_w, conv_b, A_log, Ab_log, D,
                  x_proj_w, dt_proj_w, dt_proj_b, out_proj_w):
    G = slice(96 * h, 96 * h + 96)
    f32 = np.float32
    W_in = in_proj_w.astype(f32)
    M = np.empty((3, DIN, DIM), f32)
    bconv = np.empty((3, DIN), f32)
    for k in range(3):
        M[k] = (conv_w[:, 0, k][:, None] * W_in[0::2, :]
                + conv_w[:, 1, k][:, None] * W_in[1::2, :])
        bconv[k] = (conv_w[:, 0, k] * in_proj_b[0::2]
                    + conv_w[:, 1, k] * in_proj_b[1::2])
    wm = np.empty((9, 128, DIN), f32)
    wmu = np.empty((9, 128, DH), f32)
    for s in range(3):
        for kt in range(3):
            wm[s * 3 + kt] = M[s][:, kt * 128:(kt + 1) * 128].T
            wmu[s * 3 + kt] = M[s][G, kt * 128:(kt + 1) * 128].T
    bias_int = bconv.sum(0) + conv_b
    efix = np.stack([-bconv[0], -bconv[2]])[None].astype(f32)
    efixu = efix[:, :, G].copy()
    bsil = np.zeros((128, 2), f32)
    bsil[:, 0] = bias_int[:128]
    bsil[0:64, 1] = bias_int[128:]
    A = (-np.exp(A_log)).astype(f32)
    Ab = (-np.exp(Ab_log)).astype(f32)
    seli = np.kron(np.eye(DH, dtype=f32), np.ones((1, NST), f32))
    seln = np.zeros((4, 128, 32), f32)
    for v in range(4):
        for r in range(128):
            seln[v, r, 8 * v + r // 16] = 1.0
    import ml_dtypes
    return dict(
        wm=wm,
        wmu=wmu,
        efix=efix,
        efixu=efixu,
        one=np.ones((1, 1), f32),
        bsil=bsil,
        ubias=bias_int[G].reshape(DH, 1).astype(f32),
        wxp=x_proj_w.T.astype(f32).copy(),
        wxpbc=np.concatenate(
            [x_proj_w.T[:, 24 + (np.arange(128) % 16)],
             x_proj_w.T[:, 40 + (np.arange(128) % 16)]], axis=1
        ).astype(f32).copy(),
        wdt=dt_proj_w[G].T.astype(f32).copy(),
        bsp=dt_proj_b[G].reshape(DH, 1).astype(f32),
        acol=A[G].reshape(-1).copy(),
        abcol=Ab[G].reshape(-1).copy(),
        seli=seli,
        seln=seln.astype(ml_dtypes.bfloat16),
        ddiag=np.diag(2.0 * D[G]).astype(f32),
        wouty=out_proj_w[:, G].T.astype(f32).copy(),
        woutz=out_proj_w[:, 192 + 96 * h:192 + 96 * h + 96].T.astype(f32).copy(),
    )


def kernel(x, in_proj_w, in_proj_b, conv_w, conv_b, A_log, Ab_log, D,
           x_proj_w, dt_proj_w, dt_proj_b, out_proj_w, out_proj_b):
    ab_same = bool(np.array_equal(A_log, Ab_log))
    x = np.asarray(x, np.float32)

    wargs = (in_proj_w, in_proj_b, conv_w, conv_b, A_log, Ab_log, D,
             x_proj_w, dt_proj_w, dt_proj_b, out_proj_w)
    weights = [_prep_weights(h, *[np.asarray(a, np.float32) for a in wargs])
               for h in range(2)]

    in_maps = []
    for core in range(NCORES):
        g, h = divmod(core, 2)
        xtp = np.zeros((NU, 3, 128, L + 2), np.float32)
        for u in range(NU):
            xs = x[g, u * L:(u + 1) * L, :]        # (L, 384)
            xT = np.ascontiguousarray(xs.T)        # (384, L)
            xtp[u, :, :, 1:L + 1] = xT.reshape(3, 128, L)
        m = dict(weights[h])
        m["xtp"] = xtp
        in_maps.append(m)

    nc_prog = _get_nc(ab_same)
    r = run_bass_kernel_spmd(nc_prog, in_maps, list(range(NCORES)))
    res = r.results

    out = np.empty((B, SEQ, DIM), np.float32)
    bo = np.asarray(out_proj_b, np.float32)
    for g in range(B):
        for u in range(NU):
            part = (res[2 * g]["out"][u] + res[2 * g + 1]["out"][u] + bo)
            out[g, u * L:(u + 1) * L, :] = part
    return out



# revision 8
# speedup vs baseline: 1.3839x; 1.3839x over previous
"""BiMamba layer Trainium2 kernel (8 NeuronCores, SPMD).

Sharding: 4 batch-groups x 2 d_inner-halves. Core (g, h) handles the 3
(b*f) scan units of batch g for d_inner channels G=[96h, 96h+96), both
scan directions. Each core emits a partial out-projection; the host sums
the two halves per batch and adds out_proj_b.

Key structure (all bf16 matmuls / elementwise, f32 PSUM):
  - conv+in_proj fused via 9 shifted matmuls; conv output channels are
    PERMUTED per core so the core's own 96 channels come first -> the
    SSM input u is a view xc0[0:96], no separate projection.
  - delta pre-activation matrix composed on host: Wd = dt_proj_w[G] @
    x_proj_dt  (96x192), so delta = softplus(Wd @ xc) directly
    (softplus via Exp then Ln(1+x), both in one act table).
  - B/C are produced directly in n-replicated 128-row form by repeating
    x_proj rows (wxpbc). crep_rev is a reversed AP view, not computed.
  - per k-tile (8 d-channels x 16 n-states = 128 rows):
      delta/du replicated 8->128 by SP-queue DMA broadcast (tail ks) or
      a small selection matmul (head ks, overlaps the next unit's
      front-end); daf = Exp(A * delta_rep) on ACT; dbu/hc mults on DVE;
      the two scans split between DVE and GPSIMD (Pool); n-reduction via
      per-slice seln matmuls accumulating into a [96, 512] PSUM tile.
  - 2*D*u term folded into the z-half of out_proj weights on host.
  - software pipeline: front-end of unit u+1 (conv/xproj) is emitted
    between the head and tail of unit u's scan phase.
"""
from contextlib import ExitStack

import numpy as np

import concourse.bass as bass
import concourse.tile as tile
from concourse import bacc, mybir
from concourse.bass_utils import run_bass_kernel_spmd

F32 = mybir.dt.float32
BF = mybir.dt.bfloat16
AF = mybir.ActivationFunctionType
OP = mybir.AluOpType

B, SEQ, DIM = 4, 6144, 384
L = 2048                  # per-unit sequence length
NU = 3                    # units per core
DIN, DH, NST, DTR = 192, 96, 16, 24
NK = 12                   # (DH*NST)//128 row-tiles
LC = 512                  # psum column chunk
NLC = L // LC
NCORES = 8
KP = 4                    # head k-tiles using the PE selection path
DVE_SCAN_KS = (0, 1, 6)   # k-tiles whose scans run on DVE (rest on Pool)

_NC_CACHE = {}


def _build(ab_same: bool, debug: bool = False):
    nc = bacc.Bacc("TRN2", target_bir_lowering=False, debug=False)

    def din(name, shape, dt=F32):
        return nc.dram_tensor(name, list(shape), dt, kind="ExternalInput").ap()

    xtp_d = din("xtp", (NU, 3, 128, L + 2), BF)
    wm_d = din("wm", (9, 128, DIN), BF)
    efix_d = din("efix", (1, 2, DIN), BF)
    one_d = din("one", (1, 1), BF)
    bsil_d = din("bsil", (128, 2))
    wd1_d = din("wd1", (128, DH), BF)
    wd2_d = din("wd2", (64, DH), BF)
    wxpbc1_d = din("wxpbc1", (128, 256), BF)
    wxpbc2_d = din("wxpbc2", (64, 256), BF)
    bsp_d = din("bsp", (DH, 1))
    acol_d = din("acol", (128, NK))
    abcol_d = din("abcol", (128, NK))
    seli_d = din("seli", (DH, KP * 128), BF)
    seln_d = din("seln", (4, 128, 32), BF)
    wouty_d = din("wouty", (DH, DIM), BF)
    woutz_d = din("woutz", (DH, DIM), BF)
    out_d = nc.dram_tensor("out", [NU, L, DIM], F32, kind="ExternalOutput").ap()
    dbg = {}
    if debug:
        for name, shape, dt_ in [
                ("dbg_xc0", (128, L), BF), ("dbg_xc1", (64, L), BF),
                ("dbg_delta", (DH, L), BF), ("dbg_du", (DH, L), BF),
                ("dbg_brep", (128, L), BF), ("dbg_crep", (128, L), BF),
                ("dbg_daf", (128, L), BF), ("dbg_dbu", (128, L), BF),
                ("dbg_hf", (128, L), BF), ("dbg_y", (DH, L), BF)]:
            dbg[name] = nc.dram_tensor(name, list(shape), dt_,
                                       kind="ExternalOutput").ap()

    with tile.TileContext(nc) as tc, ExitStack() as ctx:
        cp = ctx.enter_context(tc.tile_pool(name="consts", bufs=1))
        px = ctx.enter_context(tc.tile_pool(name="px", bufs=2))
        pxc = ctx.enter_context(tc.tile_pool(name="pxc", bufs=2))
        psm = ctx.enter_context(tc.tile_pool(name="psm", bufs=2))
        pesp = ctx.enter_context(tc.tile_pool(name="pesp", bufs=2))
        pbig = ctx.enter_context(tc.tile_pool(name="pbig", bufs=2))
        pout = ctx.enter_context(tc.tile_pool(name="pout", bufs=2))
        ppa = ctx.enter_context(tc.tile_pool(name="ppa", bufs=2, space="PSUM"))
        ppd = ctx.enter_context(tc.tile_pool(name="ppd", bufs=2, space="PSUM"))
        ppy = ctx.enter_context(tc.tile_pool(name="ppy", bufs=1, space="PSUM"))

        # ---- constants ----
        wm_sb = cp.tile([128, 9, DIN], BF)
        nc.sync.dma_start(wm_sb[:], wm_d.transpose([1, 0, 2]))
        efix_sb = cp.tile([1, 2, DIN], BF)
        nc.sync.dma_start(efix_sb[:], efix_d)
        one_sb = cp.tile([1, 1], BF)
        nc.sync.dma_start(one_sb[:], one_d)
        bsil_sb = cp.tile([128, 2], F32)
        nc.sync.dma_start(bsil_sb[:], bsil_d)
        wd1_sb = cp.tile([128, DH], BF)
        nc.sync.dma_start(wd1_sb[:], wd1_d)
        wd2_sb = cp.tile([64, DH], BF)
        nc.sync.dma_start(wd2_sb[:], wd2_d)
        wxpbc1_sb = cp.tile([128, 256], BF)
        nc.sync.dma_start(wxpbc1_sb[:], wxpbc1_d)
        wxpbc2_sb = cp.tile([64, 256], BF)
        nc.sync.dma_start(wxpbc2_sb[:], wxpbc2_d)
        bsp_sb = cp.tile([DH, 1], F32)
        nc.sync.dma_start(bsp_sb[:], bsp_d)
        acol_sb = cp.tile([128, NK], F32)
        nc.sync.dma_start(acol_sb[:], acol_d)
        abcol_sb = cp.tile([128, NK], F32)
        nc.sync.dma_start(abcol_sb[:], abcol_d)
        seli_sb = cp.tile([DH, KP * 128], BF)
        nc.sync.dma_start(seli_sb[:], seli_d)
        seln_sb = cp.tile([128, 4, 32], BF)
        nc.sync.dma_start(seln_sb[:], seln_d.transpose([1, 0, 2]))
        wouty_sb = cp.tile([DH, DIM], BF)
        nc.sync.dma_start(wouty_sb[:], wouty_d)
        woutz_sb = cp.tile([DH, DIM], BF)
        nc.sync.dma_start(woutz_sb[:], woutz_d)

        # per-unit persistent tiles, double buffered via pool rotation
        xts, xc0s, xc1s, breps, creps, deltas, dus = ({} for _ in range(7))

        def emit_xt_dma(u):
            xts[u] = px.tile([128, 3, L + 2], BF, name="xt")
            nc.sync.dma_start(xts[u][:], xtp_d[u].transpose([1, 0, 2]))

        def emit_frontend(u):
            """conv+in_proj+silu -> xc0/xc1; x_proj -> brep/crep;
            delta = softplus(Wd@xc); du = delta*u."""
            xt = xts[u]
            xc0 = pxc.tile([128, L], BF, name="xc0")
            xc1 = pxc.tile([64, L], BF, name="xc1")
            xc0s[u], xc1s[u] = xc0, xc1
            brep = psm.tile([128, L], BF, name="brep")
            crep = psm.tile([128, L], BF, name="crep")
            delta = psm.tile([DH, L], BF, name="delta")
            du = psm.tile([DH, L], BF, name="du")
            breps[u], creps[u] = brep, crep
            deltas[u], dus[u] = delta, du
            # conv groups
            for lc in range(NLC):
                for c0, cw, dst, bias_ap in (
                        (0, 128, xc0, bsil_sb[0:128, 0:1]),
                        (128, 64, xc1, bsil_sb[0:64, 1:2])):
                    ps = ppa.tile([128, LC], F32, tag="ppa", name="ps_conv")
                    mms = []
                    for s in range(3):
                        for kt in range(3):
                            mms.append((ps[0:cw, :],
                                        wm_sb[:, s * 3 + kt, c0:c0 + cw],
                                        xt[:, kt, s + lc * LC:s + lc * LC + LC]))
                    if lc == 0:
                        mms.append((ps[0:cw, 0:1],
                                    efix_sb[0:1, 0, c0:c0 + cw], one_sb[:]))
                    if lc == NLC - 1:
                        mms.append((ps[0:cw, LC - 1:LC],
                                    efix_sb[0:1, 1, c0:c0 + cw], one_sb[:]))
                    for i, (o, lh, rh) in enumerate(mms):
                        nc.tensor.matmul(o, lh, rh, start=(i == 0),
                                         stop=(i == len(mms) - 1))
                    nc.scalar.activation(dst[:, lc * LC:(lc + 1) * LC],
                                         ps[0:cw, :], AF.Silu, bias=bias_ap)
            # x_proj (B/C replicated) + delta
            for lc in range(NLC):
                sl = slice(lc * LC, (lc + 1) * LC)
                pdp = ppa.tile([128, LC], F32, tag="ppa", name="ps_dp")
                nc.tensor.matmul(pdp[0:DH, :], wd1_sb[:], xc0[:, sl],
                                 start=True, stop=False)
                nc.tensor.matmul(pdp[0:DH, :], wd2_sb[:], xc1[:, sl],
                                 start=False, stop=True)
                esp = pesp.tile([DH, LC], F32, name="esp", bufs=2)
                nc.scalar.activation(esp[:], pdp[0:DH, :], AF.Exp,
                                     bias=bsp_sb[:])
                nc.scalar.activation(delta[:, sl], esp[:], AF.Ln, bias=1.0)
                pbr = ppa.tile([128, LC], F32, tag="ppa", name="ps_br")
                nc.tensor.matmul(pbr[:], wxpbc1_sb[:, 0:128], xc0[:, sl],
                                 start=True, stop=False)
                nc.tensor.matmul(pbr[:], wxpbc2_sb[:, 0:128], xc1[:, sl],
                                 start=False, stop=True)
                nc.scalar.activation(brep[:, sl], pbr[:], AF.Copy)
                pcr = ppa.tile([128, LC], F32, tag="ppa", name="ps_cr")
                nc.tensor.matmul(pcr[:], wxpbc1_sb[:, 128:256], xc0[:, sl],
                                 start=True, stop=False)
                nc.tensor.matmul(pcr[:], wxpbc2_sb[:, 128:256], xc1[:, sl],
                                 start=False, stop=True)
                nc.scalar.activation(crep[:, sl], pcr[:], AF.Copy)
            # du = delta * u  (u = xc0[0:96])
            nc.vector.tensor_tensor(du[:], delta[:], xc0[0:DH, :], OP.mult)
            if debug and u == 0:
                nc.sync.dma_start(dbg["dbg_xc0"], xc0[:])
                nc.sync.dma_start(dbg["dbg_xc1"], xc1[:])
                nc.sync.dma_start(dbg["dbg_delta"], delta[:])
                nc.sync.dma_start(dbg["dbg_du"], du[:])
                nc.sync.dma_start(dbg["dbg_brep"], brep[:])
                nc.sync.dma_start(dbg["dbg_crep"], crep[:])

        def emit_scan_k(u, k, pys):
            delta, du = deltas[u], dus[u]
            brep, crep = breps[u], creps[u]
            durep = pbig.tile([128, L], BF, name="t_durep")
            nc.sync.dma_start(
                durep[:],
                du[8 * k:8 * k + 8, :].unsqueeze(1).broadcast_to([8, 16, L]))
            daf = pbig.tile([128, L], BF, name="t_daf")
            dab = daf if ab_same else pbig.tile([128, L], BF, name="t_dab")
            if k < KP:
                for lc in range(NLC):
                    sl = slice(lc * LC, (lc + 1) * LC)
                    pd = ppd.tile([128, LC], F32, tag="ppd", name="ps_sel")
                    nc.tensor.matmul(pd[:], seli_sb[:, 128 * k:128 * (k + 1)],
                                     delta[:, sl], start=True, stop=True)
                    nc.scalar.activation(daf[:, sl], pd[:], AF.Exp,
                                         scale=acol_sb[:, k:k + 1])
                    if not ab_same:
                        nc.scalar.activation(dab[:, sl], pd[:], AF.Exp,
                                             scale=abcol_sb[:, k:k + 1])
            else:
                drep = pbig.tile([128, L], BF, name="t_drep")
                nc.sync.dma_start(
                    drep[:],
                    delta[8 * k:8 * k + 8, :].unsqueeze(1)
                    .broadcast_to([8, 16, L]))
                nc.scalar.activation(daf[:], drep[:], AF.Exp,
                                     scale=acol_sb[:, k:k + 1])
                if not ab_same:
                    nc.scalar.activation(dab[:], drep[:], AF.Exp,
                                         scale=abcol_sb[:, k:k + 1])
            dbu = pbig.tile([128, L], BF, name="t_dbu")
            nc.vector.tensor_tensor(dbu[:], durep[:], brep[:], OP.mult)
            hf = pbig.tile([128, L], BF, name="t_hf")
            hb = pbig.tile([128, L], BF, name="t_hb")
            eng_s = nc.vector if k in DVE_SCAN_KS else nc.gpsimd
            eng_s.tensor_tensor_scan(hf[:], daf[:], dbu[:], 0.0,
                                     OP.mult, OP.add)
            eng_s.tensor_tensor_scan(hb[:], dab[:, ::-1], dbu[:, ::-1], 0.0,
                                     OP.mult, OP.add)
            hcf = pbig.tile([128, L], BF, name="t_hcf")
            nc.vector.tensor_tensor(hcf[:], hf[:], crep[:], OP.mult)
            hcb = pbig.tile([128, L], BF, name="t_hcb")
            nc.vector.tensor_tensor(hcb[:], hb[:], crep[:, ::-1], OP.mult)
            if debug and u == 0 and k == 0:
                nc.sync.dma_start(dbg["dbg_daf"], daf[:])
                nc.sync.dma_start(dbg["dbg_dbu"], dbu[:])
                nc.sync.dma_start(dbg["dbg_hf"], hf[:])
            kk = k % 4
            for lc in range(NLC):
                sl = slice(lc * LC, (lc + 1) * LC)
                nc.tensor.matmul(pys[lc][:], seln_sb[:, kk, :],
                                 hcf[:, sl], start=(kk == 0), stop=False)
                nc.tensor.matmul(pys[lc][:], seln_sb[:, kk, :],
                                 hcb[:, ::-1][:, sl], start=False,
                                 stop=(kk == 3))

        def emit_y_copy(u, kg, pys, y_sb):
            for lc in range(NLC):
                sl = slice(lc * LC, (lc + 1) * LC)
                nc.scalar.activation(y_sb[32 * kg:32 * kg + 32, sl],
                                     pys[lc][:], AF.Copy)

        def emit_scan_tail(u, y_sb):
            """out_proj partial + store."""
            xc0 = xc0s[u]
            if debug and u == 0:
                nc.sync.dma_start(dbg["dbg_y"], y_sb[:])
            for tq in range(L // 256):
                osb = pout.tile([128, 2, DIM], F32, name="osb")
                for j in range(2):
                    t8 = tq * 2 + j
                    sl = slice(t8 * 128, (t8 + 1) * 128)
                    po = ppa.tile([128, LC], F32, tag="ppa", name="ps_o")
                    nc.tensor.matmul(po[:, 0:DIM], y_sb[:, sl], wouty_sb[:],
                                     start=True, stop=False)
                    nc.tensor.matmul(po[:, 0:DIM], xc0[0:DH, sl], woutz_sb[:],
                                     start=False, stop=True)
                    nc.scalar.activation(osb[:, j, :], po[:, 0:DIM], AF.Copy)
                nc.sync.dma_start(
                    out_d[u, tq * 256:(tq + 1) * 256, :]
                    .rearrange("(j p) c -> p j c", p=128),
                    osb[:])

        # ---- software pipeline: FE(u+1) between kg=0 and kg=1 of SCAN(u)
        emit_xt_dma(0)
        emit_frontend(0)
        for u in range(NU):
            y_sb = psm.tile([DH, L], BF, name="y_sb")
            for kg in range(3):
                pys = [ppy.tile([32, LC], F32, tag=f"pys{lc}",
                                name=f"pys{lc}") for lc in range(NLC)]
                for kk in range(4):
                    emit_scan_k(u, 4 * kg + kk, pys)
                emit_y_copy(u, kg, pys, y_sb)
                if kg == 0 and u + 1 < NU:
                    emit_xt_dma(u + 1)
                    emit_frontend(u + 1)
            emit_scan_tail(u, y_sb)

    nc.compile()
    return nc


def _get_nc(ab_same: bool):
    if ab_same not in _NC_CACHE:
        _NC_CACHE[ab_same] = _build(ab_same)
    return _NC_CACHE[ab_same]


def _prep_weights(h, in_proj_w, in_proj_b, conv_w, conv_b, A_log, Ab_log, D,
                  x_proj_w, dt_proj_w, dt_proj_b, out_proj_w):
    import ml_dtypes
    bf = ml_dtypes.bfloat16
    f32 = np.float32
    G = np.arange(96 * h, 96 * h + 96)
    rest = np.array([d for d in range(DIN) if d not in set(G.tolist())])
    perm = np.concatenate([G, rest])

    W_in = in_proj_w.astype(f32)
    M = np.empty((3, DIN, DIM), f32)
    bconv = np.empty((3, DIN), f32)
    for k in range(3):
        M[k] = (conv_w[:, 0, k][:, None] * W_in[0::2, :]
                + conv_w[:, 1, k][:, None] * W_in[1::2, :])
        bconv[k] = (conv_w[:, 0, k] * in_proj_b[0::2]
                    + conv_w[:, 1, k] * in_proj_b[1::2])
    Mp = M[:, perm, :]
    wm = np.empty((9, 128, DIN), f32)
    for s in range(3):
        for kt in range(3):
            wm[s * 3 + kt] = Mp[s][:, kt * 128:(kt + 1) * 128].T
    bias_int = (bconv.sum(0) + conv_b)[perm]
    efix = np.stack([-bconv[0][perm], -bconv[2][perm]])[None]
    bsil = np.zeros((128, 2), f32)
    bsil[:, 0] = bias_int[:128]
    bsil[0:64, 1] = bias_int[128:]

    xp_p = x_proj_w[:, perm].astype(f32)          # (56, 192) permuted
    Wd = dt_proj_w[G].astype(f32) @ xp_p[0:DTR]   # (96, 192)
    wxpbc = np.concatenate(
        [xp_p.T[:, DTR + (np.arange(128) % NST)],
         xp_p.T[:, DTR + NST + (np.arange(128) % NST)]], axis=1)  # (192, 256)

    A = (-np.exp(A_log)).astype(f32)[G]
    Ab = (-np.exp(Ab_log)).astype(f32)[G]
    acol = np.empty((128, NK), f32)
    abcol = np.empty((128, NK), f32)
    r = np.arange(128)
    for k in range(NK):
        acol[:, k] = A[8 * k + r // NST, r % NST]
        abcol[:, k] = Ab[8 * k + r // NST, r % NST]
    seli = np.zeros((DH, KP * 128), f32)
    for k in range(KP):
        seli[8 * k + r // NST, 128 * k + r] = 1.0
    seln = np.zeros((4, 128, 32), f32)
    for kk in range(4):
        seln[kk, r, 8 * kk + r // NST] = 1.0

    wouty = out_proj_w[:, G].T.astype(f32)                    # (96, 384)
    woutz = (out_proj_w[:, DIM // 2 + G].T.astype(f32)
             + 2.0 * D[G].astype(f32)[:, None] * wouty)

    return dict(
        wm=wm.astype(bf),
        efix=efix.astype(bf),
        one=np.ones((1, 1), bf),
        bsil=bsil,
        wd1=Wd[:, 0:128].T.astype(bf).copy(),
        wd2=Wd[:, 128:192].T.astype(bf).copy(),
        wxpbc1=wxpbc[0:128].astype(bf).copy(),
        wxpbc2=wxpbc[128:192].astype(bf).copy(),
        bsp=dt_proj_b[G].reshape(DH, 1).astype(f32),
        acol=acol,
        abcol=abcol,
        seli=seli.astype(bf),
        seln=seln.astype(bf),
        wouty=wouty.astype(bf),
        woutz=woutz.astype(bf),
    )


def kernel(x, in_proj_w, in_proj_b, conv_w, conv_b, A_log, Ab_log, D,
           x_proj_w, dt_proj_w, dt_proj_b, out_proj_w, out_proj_b):
    import ml_dtypes
    bf = ml_dtypes.bfloat16
    ab_same = bool(np.array_equal(A_log, Ab_log))
    x = np.asarray(x, np.float32)

    wargs = (in_proj_w, in_proj_b, conv_w, conv_b, A_log, Ab_log, D,
             x_proj_w, dt_proj_w, dt_proj_b, out_proj_w)
    weights = [_prep_weights(h, *[np.asarray(a, np.float32) for a in wargs])
               for h in range(2)]

    xtps = []
    for g in range(B):
        xtp = np.zeros((NU, 3, 128, L + 2), bf)
        for u in range(NU):
            xs = x[g, u * L:(u + 1) * L, :]        # (L, 384)
            xT = np.ascontiguousarray(xs.T)        # (384, L)
            xtp[u, :, :, 1:L + 1] = xT.reshape(3, 128, L).astype(bf)
        xtps.append(xtp)

    in_maps = []
    for core in range(NCORES):
        g, h = divmod(core, 2)
        m = dict(weights[h])
        m["xtp"] = xtps[g]
        in_maps.append(m)

    nc_prog = _get_nc(ab_same)
    r = run_bass_kernel_spmd(nc_prog, in_maps, list(range(NCORES)))
    res = r.results

    out = np.empty((B, SEQ, DIM), np.float32)
    bo = np.asarray(out_proj_b, np.float32)
    for g in range(B):
        for u in range(NU):
            part = (res[2 * g]["out"][u] + res[2 * g + 1]["out"][u] + bo)
            out[g, u * L:(u + 1) * L, :] = part
    return out


# revision 11
# speedup vs baseline: 1.5134x; 1.0936x over previous
"""BiMamba layer Trainium2 kernel (8 NeuronCores, SPMD).

Sharding: 4 batch-groups x 2 d_inner-halves. Core (g, h) handles the 3
(b*f) scan units of batch g for d_inner channels G=[96h, 96h+96), both
scan directions. Each core emits a partial out-projection; the host sums
the two halves per batch and adds out_proj_b.

Key structure (all bf16 matmuls / elementwise, f32 PSUM):
  - conv+in_proj fused via 9 shifted matmuls; conv output channels are
    PERMUTED per core so the core's own 96 channels come first -> the
    SSM input u is a view xc0[0:96], no separate projection.
  - delta pre-activation matrix composed on host: Wd = dt_proj_w[G] @
    x_proj_dt  (96x192), so delta = softplus(Wd @ xc) directly
    (softplus via Exp then Ln(1+x), both in one act table).
  - B/C are produced directly in n-replicated 128-row form by repeating
    x_proj rows (wxpbc). crep_rev is a reversed AP view, not computed.
  - per k-tile (8 d-channels x 16 n-states = 128 rows):
      delta/du replicated 8->128 by SP-queue DMA broadcast (tail ks) or
      a small selection matmul (head ks, overlaps the next unit's
      front-end); daf = Exp(A * delta_rep) on ACT; dbu/hc mults on DVE;
      the two scans split between DVE and GPSIMD (Pool); n-reduction via
      per-slice seln matmuls accumulating into a [96, 512] PSUM tile.
  - 2*D*u term folded into the z-half of out_proj weights on host.
  - software pipeline: front-end of unit u+1 (conv/xproj) is emitted
    between the head and tail of unit u's scan phase.
"""
from contextlib import ExitStack

import numpy as np

import concourse.bass as bass
import concourse.tile as tile
from concourse import bacc, mybir
from concourse.bass_utils import run_bass_kernel_spmd

F32 = mybir.dt.float32
BF = mybir.dt.bfloat16
AF = mybir.ActivationFunctionType
OP = mybir.AluOpType

B, SEQ, DIM = 4, 6144, 384
L = 2048                  # per-unit sequence length
NU = 3                    # units per core
DIN, DH, NST, DTR = 192, 96, 16, 24
NK = 12                   # (DH*NST)//128 row-tiles
LC = 512                  # psum column chunk
NLC = L // LC
NCORES = 8
KP = 4                    # head k-tiles using the PE selection path
DVE_SCAN_KS = (0, 1, 6)   # k-tiles whose scans run on DVE (rest on Pool)

_NC_CACHE = {}


def _build(ab_same: bool, debug: bool = False):
    nc = bacc.Bacc("TRN2", target_bir_lowering=False, debug=False)

    def din(name, shape, dt=F32):
        return nc.dram_tensor(name, list(shape), dt, kind="ExternalInput").ap()

    xtp_d = din("xtp", (NU, 3, 128, L + 2), BF)
    wm_d = din("wm", (9, 128, DIN), BF)
    efix_d = din("efix", (1, 2, DIN), BF)
    one_d = din("one", (1, 1), BF)
    bsil_d = din("bsil", (128, 2))
    wd1_d = din("wd1", (128, DH), BF)
    wd2_d = din("wd2", (64, DH), BF)
    wxpbc1_d = din("wxpbc1", (128, 256), BF)
    wxpbc2_d = din("wxpbc2", (64, 256), BF)
    bsp_d = din("bsp", (DH, 1))
    acol_d = din("acol", (128, NK))
    abcol_d = din("abcol", (128, NK))
    seli_d = din("seli", (DH, KP * 128), BF)
    seln_d = din("seln", (4, 128, 32), BF)
    wouty_d = din("wouty", (DH, DIM), BF)
    woutz_d = din("woutz", (DH, DIM), BF)
    out_d = nc.dram_tensor("out", [NU, L, DIM], F32, kind="ExternalOutput").ap()
    dbg = {}
    if debug:
        for name, shape, dt_ in [
                ("dbg_xc0", (128, L), BF), ("dbg_xc1", (64, L), BF),
                ("dbg_delta", (DH, L), BF), ("dbg_du", (DH, L), BF),
                ("dbg_brep", (128, L), BF), ("dbg_crep", (128, L), BF),
                ("dbg_daf", (128, L), BF), ("dbg_dbu", (128, L), BF),
                ("dbg_hf", (128, L), BF), ("dbg_y", (DH, L), BF)]:
            dbg[name] = nc.dram_tensor(name, list(shape), dt_,
                                       kind="ExternalOutput").ap()

    with tile.TileContext(nc) as tc, ExitStack() as ctx:
        cp = ctx.enter_context(tc.tile_pool(name="consts", bufs=1))
        px = ctx.enter_context(tc.tile_pool(name="px", bufs=2))
        pxc = ctx.enter_context(tc.tile_pool(name="pxc", bufs=2))
        psm = ctx.enter_context(tc.tile_pool(name="psm", bufs=2))
        pesp = ctx.enter_context(tc.tile_pool(name="pesp", bufs=1))
        pbig = ctx.enter_context(tc.tile_pool(name="pbig", bufs=2))
        pout = ctx.enter_context(tc.tile_pool(name="pout", bufs=2))
        ppa = ctx.enter_context(tc.tile_pool(name="ppa", bufs=2, space="PSUM"))
        ppd = ctx.enter_context(tc.tile_pool(name="ppd", bufs=2, space="PSUM"))
        ppy = ctx.enter_context(tc.tile_pool(name="ppy", bufs=1, space="PSUM"))

        # ---- constants ----
        wm_sb = cp.tile([128, 9, DIN], BF)
        nc.sync.dma_start(wm_sb[:], wm_d.transpose([1, 0, 2]))
        efix_sb = cp.tile([1, 2, DIN], BF)
        nc.sync.dma_start(efix_sb[:], efix_d)
        one_sb = cp.tile([1, 1], BF)
        nc.sync.dma_start(one_sb[:], one_d)
        bsil_sb = cp.tile([128, 2], F32)
        nc.sync.dma_start(bsil_sb[:], bsil_d)
        wd1_sb = cp.tile([128, DH], BF)
        nc.sync.dma_start(wd1_sb[:], wd1_d)
        wd2_sb = cp.tile([64, DH], BF)
        nc.sync.dma_start(wd2_sb[:], wd2_d)
        wxpbc1_sb = cp.tile([128, 256], BF)
        nc.sync.dma_start(wxpbc1_sb[:], wxpbc1_d)
        wxpbc2_sb = cp.tile([64, 256], BF)
        nc.sync.dma_start(wxpbc2_sb[:], wxpbc2_d)
        bsp_sb = cp.tile([DH, 1], F32)
        nc.sync.dma_start(bsp_sb[:], bsp_d)
        acol_sb = cp.tile([128, NK], F32)
        nc.sync.dma_start(acol_sb[:], acol_d)
        abcol_sb = cp.tile([128, NK], F32)
        nc.sync.dma_start(abcol_sb[:], abcol_d)
        seli_sb = cp.tile([DH, KP * 128], BF)
        nc.sync.dma_start(seli_sb[:], seli_d)
        seln_sb = cp.tile([128, 4, 32], BF)
        nc.sync.dma_start(seln_sb[:], seln_d.transpose([1, 0, 2]))
        wouty_sb = cp.tile([DH, DIM], BF)
        nc.sync.dma_start(wouty_sb[:], wouty_d)
        woutz_sb = cp.tile([DH, DIM], BF)
        nc.sync.dma_start(woutz_sb[:], woutz_d)

        # per-unit persistent tiles, double buffered via pool rotation
        xts, xc0s, xc1s, breps, creps, deltas, dus = ({} for _ in range(7))

        def emit_xt_dma(u):
            xts[u] = px.tile([128, 3, L + 2], BF, name="xt")
            nc.sync.dma_start(xts[u][:], xtp_d[u].transpose([1, 0, 2]))

        def emit_frontend(u):
            """conv+in_proj+silu -> xc0/xc1; x_proj -> brep/crep;
            delta = softplus(Wd@xc); du = delta*u."""
            xt = xts[u]
            xc0 = pxc.tile([128, L], BF, name="xc0")
            xc1 = pxc.tile([64, L], BF, name="xc1")
            xc0s[u], xc1s[u] = xc0, xc1
            brep = psm.tile([128, L], BF, name="brep")
            crep = psm.tile([128, L], BF, name="crep")
            delta = psm.tile([DH, L], BF, name="delta")
            du = psm.tile([DH, L], BF, name="du")
            breps[u], creps[u] = brep, crep
            deltas[u], dus[u] = delta, du
            # conv groups (all silu acts contiguous -> one table region)
            for lc in range(NLC):
                for c0, cw, dst, bias_ap in (
                        (0, 128, xc0, bsil_sb[0:128, 0:1]),
                        (128, 64, xc1, bsil_sb[0:64, 1:2])):
                    ps = ppa.tile([128, LC], F32, tag="ppa", name="ps_conv")
                    mms = []
                    for s in range(3):
                        for kt in range(3):
                            mms.append((ps[0:cw, :],
                                        wm_sb[:, s * 3 + kt, c0:c0 + cw],
                                        xt[:, kt, s + lc * LC:s + lc * LC + LC]))
                    if lc == 0:
                        mms.append((ps[0:cw, 0:1],
                                    efix_sb[0:1, 0, c0:c0 + cw], one_sb[:]))
                    if lc == NLC - 1:
                        mms.append((ps[0:cw, LC - 1:LC],
                                    efix_sb[0:1, 1, c0:c0 + cw], one_sb[:]))
                    for i, (o, lh, rh) in enumerate(mms):
                        nc.tensor.matmul(o, lh, rh, start=(i == 0),
                                         stop=(i == len(mms) - 1))
                    nc.scalar.activation(dst[:, lc * LC:(lc + 1) * LC],
                                         ps[0:cw, :], AF.Silu, bias=bias_ap)
            # delta pre-activation: batch the 4 Exp chunks, then one Ln
            esp = pesp.tile([DH, L], F32, name="esp")
            for lc in range(NLC):
                sl = slice(lc * LC, (lc + 1) * LC)
                pdp = ppa.tile([128, LC], F32, tag="ppa", name="ps_dp")
                nc.tensor.matmul(pdp[0:DH, :], wd1_sb[:], xc0[:, sl],
                                 start=True, stop=False)
                nc.tensor.matmul(pdp[0:DH, :], wd2_sb[:], xc1[:, sl],
                                 start=False, stop=True)
                nc.scalar.activation(esp[:, sl], pdp[0:DH, :], AF.Exp,
                                     bias=bsp_sb[:])
            nc.scalar.activation(delta[:], esp[:], AF.Ln, bias=1.0)
            # x_proj (B/C replicated)
            for lc in range(NLC):
                sl = slice(lc * LC, (lc + 1) * LC)
                pbr = ppa.tile([128, LC], F32, tag="ppa", name="ps_br")
                nc.tensor.matmul(pbr[:], wxpbc1_sb[:, 0:128], xc0[:, sl],
                                 start=True, stop=False)
                nc.tensor.matmul(pbr[:], wxpbc2_sb[:, 0:128], xc1[:, sl],
                                 start=False, stop=True)
                nc.scalar.activation(brep[:, sl], pbr[:], AF.Copy)
                pcr = ppa.tile([128, LC], F32, tag="ppa", name="ps_cr")
                nc.tensor.matmul(pcr[:], wxpbc1_sb[:, 128:256], xc0[:, sl],
                                 start=True, stop=False)
                nc.tensor.matmul(pcr[:], wxpbc2_sb[:, 128:256], xc1[:, sl],
                                 start=False, stop=True)
                nc.scalar.activation(crep[:, sl], pcr[:], AF.Copy)
            # du = delta * u  (u = xc0[0:96])
            nc.vector.tensor_tensor(du[:], delta[:], xc0[0:DH, :], OP.mult)
            if debug and u == 0:
                nc.sync.dma_start(dbg["dbg_xc0"], xc0[:])
                nc.sync.dma_start(dbg["dbg_xc1"], xc1[:])
                nc.sync.dma_start(dbg["dbg_delta"], delta[:])
                nc.sync.dma_start(dbg["dbg_du"], du[:])
                nc.sync.dma_start(dbg["dbg_brep"], brep[:])
                nc.sync.dma_start(dbg["dbg_crep"], crep[:])

        def emit_scan_k(u, k, pys):
            delta, du = deltas[u], dus[u]
            brep, crep = breps[u], creps[u]
            durep = pbig.tile([128, L], BF, name="t_durep")
            nc.sync.dma_start(
                durep[:],
                du[8 * k:8 * k + 8, :].unsqueeze(1).broadcast_to([8, 16, L]))
            daf = pbig.tile([128, L], BF, name="t_daf")
            dab = daf if ab_same else pbig.tile([128, L], BF, name="t_dab")
            if k < KP:
                for lc in range(NLC):
                    sl = slice(lc * LC, (lc + 1) * LC)
                    pd = ppd.tile([128, LC], F32, tag="ppd", name="ps_sel")
                    nc.tensor.matmul(pd[:], seli_sb[:, 128 * k:128 * (k + 1)],
                                     delta[:, sl], start=True, stop=True)
                    nc.scalar.activation(daf[:, sl], pd[:], AF.Exp,
                                         scale=acol_sb[:, k:k + 1])
                    if not ab_same:
                        nc.scalar.activation(dab[:, sl], pd[:], AF.Exp,
                                             scale=abcol_sb[:, k:k + 1])
            else:
                drep = pbig.tile([128, L], BF, name="t_drep")
                nc.sync.dma_start(
                    drep[:],
                    delta[8 * k:8 * k + 8, :].unsqueeze(1)
                    .broadcast_to([8, 16, L]))
                nc.scalar.activation(daf[:], drep[:], AF.Exp,
                                     scale=acol_sb[:, k:k + 1])
                if not ab_same:
                    nc.scalar.activation(dab[:], drep[:], AF.Exp,
                                         scale=abcol_sb[:, k:k + 1])
            dbu = pbig.tile([128, L], BF, name="t_dbu")
            nc.vector.tensor_tensor(dbu[:], durep[:], brep[:], OP.mult)
            hf = pbig.tile([128, L], BF, name="t_hf")
            hb = pbig.tile([128, L], BF, name="t_hb")
            eng_s = nc.vector if k in DVE_SCAN_KS else nc.gpsimd
            eng_s.tensor_tensor_scan(hf[:], daf[:], dbu[:], 0.0,
                                     OP.mult, OP.add)
            eng_s.tensor_tensor_scan(hb[:], dab[:, ::-1], dbu[:, ::-1], 0.0,
                                     OP.mult, OP.add)
            hcf = pbig.tile([128, L], BF, name="t_hcf")
            nc.vector.tensor_tensor(hcf[:], hf[:], crep[:], OP.mult)
            hcb = pbig.tile([128, L], BF, name="t_hcb")
            nc.vector.tensor_tensor(hcb[:], hb[:], crep[:, ::-1], OP.mult)
            if debug and u == 0 and k == 0:
                nc.sync.dma_start(dbg["dbg_daf"], daf[:])
                nc.sync.dma_start(dbg["dbg_dbu"], dbu[:])
                nc.sync.dma_start(dbg["dbg_hf"], hf[:])
            kk = k % 4
            for lc in range(NLC):
                sl = slice(lc * LC, (lc + 1) * LC)
                nc.tensor.matmul(pys[lc][:], seln_sb[:, kk, :],
                                 hcf[:, sl], start=(kk == 0), stop=False)
                nc.tensor.matmul(pys[lc][:], seln_sb[:, kk, :],
                                 hcb[:, ::-1][:, sl], start=False,
                                 stop=(kk == 3))

        def emit_y_copy(u, kg, pys, y_sb):
            for lc in range(NLC):
                sl = slice(lc * LC, (lc + 1) * LC)
                nc.scalar.activation(y_sb[32 * kg:32 * kg + 32, sl],
                                     pys[lc][:], AF.Copy)

        def emit_scan_tail(u, y_sb):
            """out_proj partial + store."""
            xc0 = xc0s[u]
            if debug and u == 0:
                nc.sync.dma_start(dbg["dbg_y"], y_sb[:])
            for tq in range(L // 256):
                osb = pout.tile([128, 2, DIM], F32, name="osb")
                for j in range(2):
                    t8 = tq * 2 + j
                    sl = slice(t8 * 128, (t8 + 1) * 128)
                    po = ppa.tile([128, LC], F32, tag="ppa", name="ps_o")
                    nc.tensor.matmul(po[:, 0:DIM], y_sb[:, sl], wouty_sb[:],
                                     start=True, stop=False)
                    nc.tensor.matmul(po[:, 0:DIM], xc0[0:DH, sl], woutz_sb[:],
                                     start=False, stop=True)
                    nc.scalar.activation(osb[:, j, :], po[:, 0:DIM], AF.Copy)
                nc.sync.dma_start(
                    out_d[u, tq * 256:(tq + 1) * 256, :]
                    .rearrange("(j p) c -> p j c", p=128),
                    osb[:])

        # ---- software pipeline: FE(u+1) between kg=0 and kg=1 of SCAN(u)
        emit_xt_dma(0)
        emit_frontend(0)
        for u in range(NU):
            y_sb = psm.tile([DH, L], BF, name="y_sb")
            for kg in range(3):
                pys = [ppy.tile([32, LC], F32, tag=f"pys{lc}",
                                name=f"pys{lc}") for lc in range(NLC)]
                for kk in range(4):
                    emit_scan_k(u, 4 * kg + kk, pys)
                emit_y_copy(u, kg, pys, y_sb)
                if kg == 0 and u + 1 < NU:
                    emit_xt_dma(u + 1)
                    emit_frontend(u + 1)
            emit_scan_tail(u, y_sb)

    nc.compile()
    return nc


def _get_nc(ab_same: bool):
    if ab_same not in _NC_CACHE:
        _NC_CACHE[ab_same] = _build(ab_same)
    return _NC_CACHE[ab_same]


def _prep_weights(h, in_proj_w, in_proj_b, conv_w, conv_b, A_log, Ab_log, D,
                  x_proj_w, dt_proj_w, dt_proj_b, out_proj_w):
    import ml_dtypes
    bf = ml_dtypes.bfloat16
    f32 = np.float32
    G = np.arange(96 * h, 96 * h + 96)
    rest = np.array([d for d in range(DIN) if d not in set(G.tolist())])
    perm = np.concatenate([G, rest])

    W_in = in_proj_w.astype(f32)
    M = np.empty((3, DIN, DIM), f32)
    bconv = np.empty((3, DIN), f32)
    for k in range(3):
        M[k] = (conv_w[:, 0, k][:, None] * W_in[0::2, :]
                + conv_w[:, 1, k][:, None] * W_in[1::2, :])
        bconv[k] = (conv_w[:, 0, k] * in_proj_b[0::2]
                    + conv_w[:, 1, k] * in_proj_b[1::2])
    Mp = M[:, perm, :]
    wm = np.empty((9, 128, DIN), f32)
    for s in range(3):
        for kt in range(3):
            wm[s * 3 + kt] = Mp[s][:, kt * 128:(kt + 1) * 128].T
    bias_int = (bconv.sum(0) + conv_b)[perm]
    efix = np.stack([-bconv[0][perm], -bconv[2][perm]])[None]
    bsil = np.zeros((128, 2), f32)
    bsil[:, 0] = bias_int[:128]
    bsil[0:64, 1] = bias_int[128:]

    xp_p = x_proj_w[:, perm].astype(f32)          # (56, 192) permuted
    Wd = dt_proj_w[G].astype(f32) @ xp_p[0:DTR]   # (96, 192)
    wxpbc = np.concatenate(
        [xp_p.T[:, DTR + (np.arange(128) % NST)],
         xp_p.T[:, DTR + NST + (np.arange(128) % NST)]], axis=1)  # (192, 256)

    A = (-np.exp(A_log)).astype(f32)[G]
    Ab = (-np.exp(Ab_log)).astype(f32)[G]
    acol = np.empty((128, NK), f32)
    abcol = np.empty((128, NK), f32)
    r = np.arange(128)
    for k in range(NK):
        acol[:, k] = A[8 * k + r // NST, r % NST]
        abcol[:, k] = Ab[8 * k + r // NST, r % NST]
    seli = np.zeros((DH, KP * 128), f32)
    for k in range(KP):
        seli[8 * k + r // NST, 128 * k + r] = 1.0
    seln = np.zeros((4, 128, 32), f32)
    for kk in range(4):
        seln[kk, r, 8 * kk + r // NST] = 1.0

    wouty = out_proj_w[:, G].T.astype(f32)                    # (96, 384)
    woutz = (out_proj_w[:, DIM // 2 + G].T.astype(f32)
             + 2.0 * D[G].astype(f32)[:, None] * wouty)

    return dict(
        wm=wm.astype(bf),
        efix=efix.astype(bf),
        one=np.ones((1, 1), bf),
        bsil=bsil,
        wd1=Wd[:, 0:128].T.astype(bf).copy(),
        wd2=Wd[:, 128:192].T.astype(bf).copy(),
        wxpbc1=wxpbc[0:128].astype(bf).copy(),
        wxpbc2=wxpbc[128:192].astype(bf).copy(),
        bsp=dt_proj_b[G].reshape(DH, 1).astype(f32),
        acol=acol,
        abcol=abcol,
        seli=seli.astype(bf),
        seln=seln.astype(bf),
        wouty=wouty.astype(bf),
        woutz=woutz.astype(bf),
    )


def kernel(x, in_proj_w, in_proj_b, conv_w, conv_b, A_log, Ab_log, D,
           x_proj_w, dt_proj_w, dt_proj_b, out_proj_w, out_proj_b):
    import ml_dtypes
    bf = ml_dtypes.bfloat16
    ab_same = bool(np.array_equal(A_log, Ab_log))
    x = np.asarray(x, np.float32)

    wargs = (in_proj_w, in_proj_b, conv_w, conv_b, A_log, Ab_log, D,
             x_proj_w, dt_proj_w, dt_proj_b, out_proj_w)
    weights = [_prep_weights(h, *[np.asarray(a, np.float32) for a in wargs])
               for h in range(2)]

    xtps = []
    for g in range(B):
        xtp = np.zeros((NU, 3, 128, L + 2), bf)
        for u in range(NU):
            xs = x[g, u * L:(u + 1) * L, :]        # (L, 384)
            xT = np.ascontiguousarray(xs.T)        # (384, L)
            xtp[u, :, :, 1:L + 1] = xT.reshape(3, 128, L).astype(bf)
        xtps.append(xtp)

    in_maps = []
    for core in range(NCORES):
        g, h = divmod(core, 2)
        m = dict(weights[h])
        m["xtp"] = xtps[g]
        in_maps.append(m)

    nc_prog = _get_nc(ab_same)
    r = run_bass_kernel_spmd(nc_prog, in_maps, list(range(NCORES)))
    res = r.results

    out = np.empty((B, SEQ, DIM), np.float32)
    bo = np.asarray(out_proj_b, np.float32)
    for g in range(B):
        for u in range(NU):
            part = (res[2 * g]["out"][u] + res[2 * g + 1]["out"][u] + bo)
            out[g, u * L:(u + 1) * L, :] = part
    return out


# revision 16
# speedup vs baseline: 1.5551x; 1.0275x over previous
"""BiMamba layer Trainium2 kernel (8 NeuronCores, SPMD).

Sharding: 4 batch-groups x 2 d_inner-halves. Core (g, h) handles the 3
(b*f) scan units of batch g for d_inner channels G=[96h, 96h+96), both
scan directions. Each core emits a partial out-projection; the host sums
the two halves per batch and adds out_proj_b.

Key structure (all bf16 matmuls / elementwise, f32 PSUM):
  - conv+in_proj fused via 9 shifted matmuls; conv output channels are
    PERMUTED per core so the core's own 96 channels come first -> the
    SSM input u is a view xc0[0:96], no separate projection.
  - delta pre-activation matrix composed on host: Wd = dt_proj_w[G] @
    x_proj_dt  (96x192), so delta = softplus(Wd @ xc) directly
    (softplus via Exp then Ln(1+x), both in one act table).
  - B/C are produced directly in n-replicated 128-row form by repeating
    x_proj rows (wxpbc). crep_rev is a reversed AP view, not computed.
  - per k-tile (8 d-channels x 16 n-states = 128 rows):
      delta/du replicated 8->128 by SP-queue DMA broadcast (tail ks) or
      a small selection matmul (head ks, overlaps the next unit's
      front-end); daf = Exp(A * delta_rep) on ACT; dbu/hc mults on DVE;
      the two scans split between DVE and GPSIMD (Pool); n-reduction via
      per-slice seln matmuls accumulating into a [96, 512] PSUM tile.
  - 2*D*u term folded into the z-half of out_proj weights on host.
  - software pipeline: front-end of unit u+1 (conv/xproj) is emitted
    between the head and tail of unit u's scan phase.
"""
from contextlib import ExitStack

import numpy as np

import concourse.bass as bass
import concourse.tile as tile
from concourse import bacc, mybir
from concourse.bass_utils import run_bass_kernel_spmd

F32 = mybir.dt.float32
BF = mybir.dt.bfloat16
AF = mybir.ActivationFunctionType
OP = mybir.AluOpType

B, SEQ, DIM = 4, 6144, 384
L = 2048                  # per-unit sequence length
NU = 3                    # units per core
DIN, DH, NST, DTR = 192, 96, 16, 24
NK = 12                   # (DH*NST)//128 row-tiles
LC = 512                  # psum column chunk
NLC = L // LC
NCORES = 8
KP = 4                    # head k-tiles using the PE selection path
DVE_SCAN_KS = (0, 1, 6)   # k-tiles whose scans run on DVE (rest on Pool)

_NC_CACHE = {}


def _build(ab_same: bool, debug: bool = False):
    nc = bacc.Bacc("TRN2", target_bir_lowering=False, debug=False)

    def din(name, shape, dt=F32):
        return nc.dram_tensor(name, list(shape), dt, kind="ExternalInput").ap()

    xtp_d = din("xtp", (NU, 3, 128, L + 2), BF)
    wm_d = din("wm", (9, 128, DIN), BF)
    efix_d = din("efix", (1, 2, DIN), BF)
    one_d = din("one", (1, 1), BF)
    bsil_d = din("bsil", (128, 2))
    wd1_d = din("wd1", (128, DH), BF)
    wd2_d = din("wd2", (64, DH), BF)
    wxpbc1_d = din("wxpbc1", (128, 256), BF)
    wxpbc2_d = din("wxpbc2", (64, 256), BF)
    bsp_d = din("bsp", (DH, 1))
    acol_d = din("acol", (128, NK))
    abcol_d = din("abcol", (128, NK))
    seli_d = din("seli", (DH, KP * 128), BF)
    seln_d = din("seln", (NK, 128, DH), BF)
    wouty_d = din("wouty", (DH, DIM), BF)
    woutz_d = din("woutz", (DH, DIM), BF)
    out_d = nc.dram_tensor("out", [NU, L, DIM], F32, kind="ExternalOutput").ap()
    dbg = {}
    if debug:
        for name, shape, dt_ in [
                ("dbg_xc0", (128, L), BF), ("dbg_xc1", (64, L), BF),
                ("dbg_delta", (DH, L), BF), ("dbg_du", (DH, L), BF),
                ("dbg_brep", (128, L), BF), ("dbg_crep", (128, L), BF),
                ("dbg_daf", (128, L), BF), ("dbg_dbu", (128, L), BF),
                ("dbg_hf", (128, L), BF), ("dbg_y", (DH, L), BF)]:
            dbg[name] = nc.dram_tensor(name, list(shape), dt_,
                                       kind="ExternalOutput").ap()

    with tile.TileContext(nc) as tc, ExitStack() as ctx:
        cp = ctx.enter_context(tc.tile_pool(name="consts", bufs=1))
        px = ctx.enter_context(tc.tile_pool(name="px", bufs=2))
        pxc = ctx.enter_context(tc.tile_pool(name="pxc", bufs=2))
        psm = ctx.enter_context(tc.tile_pool(name="psm", bufs=2))
        pesp = ctx.enter_context(tc.tile_pool(name="pesp", bufs=1))
        pbig = ctx.enter_context(tc.tile_pool(name="pbig", bufs=2))
        pout = ctx.enter_context(tc.tile_pool(name="pout", bufs=2))
        ppa = ctx.enter_context(tc.tile_pool(name="ppa", bufs=2, space="PSUM"))
        ppd = ctx.enter_context(tc.tile_pool(name="ppd", bufs=2, space="PSUM"))
        ppy = ctx.enter_context(tc.tile_pool(name="ppy", bufs=1, space="PSUM"))

        # ---- constants ----
        wm_sb = cp.tile([128, 9, DIN], BF)
        nc.sync.dma_start(wm_sb[:], wm_d.transpose([1, 0, 2]))
        efix_sb = cp.tile([1, 2, DIN], BF)
        nc.sync.dma_start(efix_sb[:], efix_d)
        one_sb = cp.tile([1, 1], BF)
        nc.sync.dma_start(one_sb[:], one_d)
        bsil_sb = cp.tile([128, 2], F32)
        nc.sync.dma_start(bsil_sb[:], bsil_d)
        wd1_sb = cp.tile([128, DH], BF)
        nc.sync.dma_start(wd1_sb[:], wd1_d)
        wd2_sb = cp.tile([64, DH], BF)
        nc.sync.dma_start(wd2_sb[:], wd2_d)
        wxpbc1_sb = cp.tile([128, 256], BF)
        nc.sync.dma_start(wxpbc1_sb[:], wxpbc1_d)
        wxpbc2_sb = cp.tile([64, 256], BF)
        nc.sync.dma_start(wxpbc2_sb[:], wxpbc2_d)
        bsp_sb = cp.tile([DH, 1], F32)
        nc.sync.dma_start(bsp_sb[:], bsp_d)
        acol_sb = cp.tile([128, NK], F32)
        nc.sync.dma_start(acol_sb[:], acol_d)
        abcol_sb = cp.tile([128, NK], F32)
        nc.sync.dma_start(abcol_sb[:], abcol_d)
        seli_sb = cp.tile([DH, KP * 128], BF)
        nc.sync.dma_start(seli_sb[:], seli_d)
        seln_sb = cp.tile([128, NK, DH], BF)
        nc.sync.dma_start(seln_sb[:], seln_d.transpose([1, 0, 2]))
        wouty_sb = cp.tile([DH, DIM], BF)
        nc.sync.dma_start(wouty_sb[:], wouty_d)
        woutz_sb = cp.tile([DH, DIM], BF)
        nc.sync.dma_start(woutz_sb[:], woutz_d)

        # per-unit persistent tiles, double buffered via pool rotation
        xts, xc0s, xc1s, breps, creps, deltas, dus = ({} for _ in range(7))

        def emit_xt_dma(u):
            xts[u] = px.tile([128, 3, L + 2], BF, name="xt")
            nc.sync.dma_start(xts[u][:], xtp_d[u].transpose([1, 0, 2]))

        def emit_frontend(u):
            """conv+in_proj+silu -> xc0/xc1; x_proj -> brep/crep;
            delta = softplus(Wd@xc); du = delta*u."""
            xt = xts[u]
            xc0 = pxc.tile([128, L], BF, name="xc0")
            xc1 = pxc.tile([64, L], BF, name="xc1")
            xc0s[u], xc1s[u] = xc0, xc1
            brep = psm.tile([128, L], BF, name="brep")
            crep = psm.tile([128, L], BF, name="crep")
            delta = psm.tile([DH, L], BF, name="delta")
            du = psm.tile([DH, L], BF, name="du")
            breps[u], creps[u] = brep, crep
            deltas[u], dus[u] = delta, du
            # conv groups (all silu acts contiguous -> one table region)
            for lc in range(NLC):
                for c0, cw, dst, bias_ap in (
                        (0, 128, xc0, bsil_sb[0:128, 0:1]),
                        (128, 64, xc1, bsil_sb[0:64, 1:2])):
                    ps = ppa.tile([128, LC], F32, tag="ppa", name="ps_conv")
                    mms = []
                    for s in range(3):
                        for kt in range(3):
                            mms.append((ps[0:cw, :],
                                        wm_sb[:, s * 3 + kt, c0:c0 + cw],
                                        xt[:, kt, s + lc * LC:s + lc * LC + LC]))
                    if lc == 0:
                        mms.append((ps[0:cw, 0:1],
                                    efix_sb[0:1, 0, c0:c0 + cw], one_sb[:]))
                    if lc == NLC - 1:
                        mms.append((ps[0:cw, LC - 1:LC],
                                    efix_sb[0:1, 1, c0:c0 + cw], one_sb[:]))
                    for i, (o, lh, rh) in enumerate(mms):
                        nc.tensor.matmul(o, lh, rh, start=(i == 0),
                                         stop=(i == len(mms) - 1))
                    nc.scalar.activation(dst[:, lc * LC:(lc + 1) * LC],
                                         ps[0:cw, :], AF.Silu, bias=bias_ap)
            # delta pre-activation: batch the 4 Exp chunks, then one Ln
            esp = pesp.tile([DH, L], F32, name="esp")
            for lc in range(NLC):
                sl = slice(lc * LC, (lc + 1) * LC)
                pdp = ppa.tile([128, LC], F32, tag="ppa", name="ps_dp")
                nc.tensor.matmul(pdp[0:DH, :], wd1_sb[:], xc0[:, sl],
                                 start=True, stop=False)
                nc.tensor.matmul(pdp[0:DH, :], wd2_sb[:], xc1[:, sl],
                                 start=False, stop=True)
                nc.scalar.activation(esp[:, sl], pdp[0:DH, :], AF.Exp,
                                     bias=bsp_sb[:])
            nc.scalar.activation(delta[:], esp[:], AF.Ln, bias=1.0)
            # x_proj (B/C replicated)
            for lc in range(NLC):
                sl = slice(lc * LC, (lc + 1) * LC)
                pbr = ppa.tile([128, LC], F32, tag="ppa", name="ps_br")
                nc.tensor.matmul(pbr[:], wxpbc1_sb[:, 0:128], xc0[:, sl],
                                 start=True, stop=False)
                nc.tensor.matmul(pbr[:], wxpbc2_sb[:, 0:128], xc1[:, sl],
                                 start=False, stop=True)
                nc.scalar.activation(brep[:, sl], pbr[:], AF.Copy)
                pcr = ppa.tile([128, LC], F32, tag="ppa", name="ps_cr")
                nc.tensor.matmul(pcr[:], wxpbc1_sb[:, 128:256], xc0[:, sl],
                                 start=True, stop=False)
                nc.tensor.matmul(pcr[:], wxpbc2_sb[:, 128:256], xc1[:, sl],
                                 start=False, stop=True)
                nc.scalar.activation(crep[:, sl], pcr[:], AF.Copy)
            # du = delta * u  (u = xc0[0:96])
            nc.vector.tensor_tensor(du[:], delta[:], xc0[0:DH, :], OP.mult)
            if debug and u == 0:
                nc.sync.dma_start(dbg["dbg_xc0"], xc0[:])
                nc.sync.dma_start(dbg["dbg_xc1"], xc1[:])
                nc.sync.dma_start(dbg["dbg_delta"], delta[:])
                nc.sync.dma_start(dbg["dbg_du"], du[:])
                nc.sync.dma_start(dbg["dbg_brep"], brep[:])
                nc.sync.dma_start(dbg["dbg_crep"], crep[:])

        def emit_scan_k(u, k, pys):
            delta, du = deltas[u], dus[u]
            brep, crep = breps[u], creps[u]
            durep = pbig.tile([128, L], BF, name="t_durep")
            nc.sync.dma_start(
                durep[:],
                du[8 * k:8 * k + 8, :].unsqueeze(1).broadcast_to([8, 16, L]))
            daf = pbig.tile([128, L], BF, name="t_daf")
            dab = daf if ab_same else pbig.tile([128, L], BF, name="t_dab")
            if k < KP:
                for lc in range(NLC):
                    sl = slice(lc * LC, (lc + 1) * LC)
                    pd = ppd.tile([128, LC], F32, tag="ppd", name="ps_sel")
                    nc.tensor.matmul(pd[:], seli_sb[:, 128 * k:128 * (k + 1)],
                                     delta[:, sl], start=True, stop=True)
                    nc.scalar.activation(daf[:, sl], pd[:], AF.Exp,
                                         scale=acol_sb[:, k:k + 1])
                    if not ab_same:
                        nc.scalar.activation(dab[:, sl], pd[:], AF.Exp,
                                             scale=abcol_sb[:, k:k + 1])
            else:
                drep = pbig.tile([128, L], BF, name="t_drep")
                nc.sync.dma_start(
                    drep[:],
                    delta[8 * k:8 * k + 8, :].unsqueeze(1)
                    .broadcast_to([8, 16, L]))
                nc.scalar.activation(daf[:], drep[:], AF.Exp,
                                     scale=acol_sb[:, k:k + 1])
                if not ab_same:
                    nc.scalar.activation(dab[:], drep[:], AF.Exp,
                                         scale=abcol_sb[:, k:k + 1])
            dbu = pbig.tile([128, L], BF, name="t_dbu")
            nc.vector.tensor_tensor(dbu[:], durep[:], brep[:], OP.mult)
            hf = pbig.tile([128, L], BF, name="t_hf")
            hb = pbig.tile([128, L], BF, name="t_hb")
            eng_s = nc.vector if k in DVE_SCAN_KS else nc.gpsimd
            eng_s.tensor_tensor_scan(hf[:], daf[:], dbu[:], 0.0,
                                     OP.mult, OP.add)
            eng_s.tensor_tensor_scan(hb[:], dab[:, ::-1], dbu[:, ::-1], 0.0,
                                     OP.mult, OP.add)
            hcf = pbig.tile([128, L], BF, name="t_hcf")
            nc.vector.tensor_tensor(hcf[:], hf[:], crep[:], OP.mult)
            hcb = pbig.tile([128, L], BF, name="t_hcb")
            nc.vector.tensor_tensor(hcb[:], hb[:], crep[:, ::-1], OP.mult)
            if debug and u == 0 and k == 0:
                nc.sync.dma_start(dbg["dbg_daf"], daf[:])
                nc.sync.dma_start(dbg["dbg_dbu"], dbu[:])
                nc.sync.dma_start(dbg["dbg_hf"], hf[:])
            for lc in range(NLC):
                sl = slice(lc * LC, (lc + 1) * LC)
                nc.tensor.matmul(pys[lc][:], seln_sb[:, k, :],
                                 hcf[:, sl], start=(k == 0), stop=False)
                nc.tensor.matmul(pys[lc][:], seln_sb[:, k, :],
                                 hcb[:, ::-1][:, sl], start=False,
                                 stop=(k == NK - 1))

        def emit_scan_tail(u, pys):
            """y copy + out_proj partial + store."""
            xc0 = xc0s[u]
            y_sb = psm.tile([DH, L], BF, name="y_sb")
            for lc in range(NLC):
                sl = slice(lc * LC, (lc + 1) * LC)
                nc.scalar.activation(y_sb[:, sl], pys[lc][:], AF.Copy)
            if debug and u == 0:
                nc.sync.dma_start(dbg["dbg_y"], y_sb[:])
            for tq in range(L // 256):
                osb = pout.tile([128, 2, DIM], F32, name="osb")
                for j in range(2):
                    t8 = tq * 2 + j
                    sl = slice(t8 * 128, (t8 + 1) * 128)
                    po = ppa.tile([128, LC], F32, tag="ppa", name="ps_o")
                    nc.tensor.matmul(po[:, 0:DIM], y_sb[:, sl], wouty_sb[:],
                                     start=True, stop=False)
                    nc.tensor.matmul(po[:, 0:DIM], xc0[0:DH, sl], woutz_sb[:],
                                     start=False, stop=True)
                    nc.scalar.activation(osb[:, j, :], po[:, 0:DIM], AF.Copy)
                nc.sync.dma_start(
                    out_d[u, tq * 256:(tq + 1) * 256, :]
                    .rearrange("(j p) c -> p j c", p=128),
                    osb[:])

        # ---- software pipeline: FE(u+1) between head ks and tail of SCAN(u)
        emit_xt_dma(0)
        emit_frontend(0)
        for u in range(NU):
            pys = [ppy.tile([DH, LC], F32, tag=f"pys{lc}",
                            name=f"pys{lc}") for lc in range(NLC)]
            for k in range(KP):
                emit_scan_k(u, k, pys)
            if u + 1 < NU:
                emit_xt_dma(u + 1)
                emit_frontend(u + 1)
            for k in range(KP, NK):
                emit_scan_k(u, k, pys)
            emit_scan_tail(u, pys)

    nc.compile()
    return nc


def _get_nc(ab_same: bool):
    if ab_same not in _NC_CACHE:
        _NC_CACHE[ab_same] = _build(ab_same)
    return _NC_CACHE[ab_same]


def _prep_weights(h, in_proj_w, in_proj_b, conv_w, conv_b, A_log, Ab_log, D,
                  x_proj_w, dt_proj_w, dt_proj_b, out_proj_w):
    import ml_dtypes
    bf = ml_dtypes.bfloat16
    f32 = np.float32
    G = np.arange(96 * h, 96 * h + 96)
    rest = np.array([d for d in range(DIN) if d not in set(G.tolist())])
    perm = np.concatenate([G, rest])

    W_in = in_proj_w.astype(f32)
    M = np.empty((3, DIN, DIM), f32)
    bconv = np.empty((3, DIN), f32)
    for k in range(3):
        M[k] = (conv_w[:, 0, k][:, None] * W_in[0::2, :]
                + conv_w[:, 1, k][:, None] * W_in[1::2, :])
        bconv[k] = (conv_w[:, 0, k] * in_proj_b[0::2]
                    + conv_w[:, 1, k] * in_proj_b[1::2])
    Mp = M[:, perm, :]
    wm = np.empty((9, 128, DIN), f32)
    for s in range(3):
        for kt in range(3):
            wm[s * 3 + kt] = Mp[s][:, kt * 128:(kt + 1) * 128].T
    bias_int = (bconv.sum(0) + conv_b)[perm]
    efix = np.stack([-bconv[0][perm], -bconv[2][perm]])[None]
    bsil = np.zeros((128, 2), f32)
    bsil[:, 0] = bias_int[:128]
    bsil[0:64, 1] = bias_int[128:]

    xp_p = x_proj_w[:, perm].astype(f32)          # (56, 192) permuted
    Wd = dt_proj_w[G].astype(f32) @ xp_p[0:DTR]   # (96, 192)
    wxpbc = np.concatenate(
        [xp_p.T[:, DTR + (np.arange(128) % NST)],
         xp_p.T[:, DTR + NST + (np.arange(128) % NST)]], axis=1)  # (192, 256)

    A = (-np.exp(A_log)).astype(f32)[G]
    Ab = (-np.exp(Ab_log)).astype(f32)[G]
    acol = np.empty((128, NK), f32)
    abcol = np.empty((128, NK), f32)
    r = np.arange(128)
    for k in range(NK):
        acol[:, k] = A[8 * k + r // NST, r % NST]
        abcol[:, k] = Ab[8 * k + r // NST, r % NST]
    seli = np.zeros((DH, KP * 128), f32)
    for k in range(KP):
        seli[8 * k + r // NST, 128 * k + r] = 1.0
    seln = np.zeros((NK, 128, DH), f32)
    for k in range(NK):
        seln[k, r, 8 * k + r // NST] = 1.0

    wouty = out_proj_w[:, G].T.astype(f32)                    # (96, 384)
    woutz = (out_proj_w[:, DIM // 2 + G].T.astype(f32)
             + 2.0 * D[G].astype(f32)[:, None] * wouty)

    return dict(
        wm=wm.astype(bf),
        efix=efix.astype(bf),
        one=np.ones((1, 1), bf),
        bsil=bsil,
        wd1=Wd[:, 0:128].T.astype(bf).copy(),
        wd2=Wd[:, 128:192].T.astype(bf).copy(),
        wxpbc1=wxpbc[0:128].astype(bf).copy(),
        wxpbc2=wxpbc[128:192].astype(bf).copy(),
        bsp=dt_proj_b[G].reshape(DH, 1).astype(f32),
        acol=acol,
        abcol=abcol,
        seli=seli.astype(bf),
        seln=seln.astype(bf),
        wouty=wouty.astype(bf),
        woutz=woutz.astype(bf),
    )


def kernel(x, in_proj_w, in_proj_b, conv_w, conv_b, A_log, Ab_log, D,
           x_proj_w, dt_proj_w, dt_proj_b, out_proj_w, out_proj_b):
    import ml_dtypes
    bf = ml_dtypes.bfloat16
    ab_same = bool(np.array_equal(A_log, Ab_log))
    x = np.asarray(x, np.float32)

    wargs = (in_proj_w, in_proj_b, conv_w, conv_b, A_log, Ab_log, D,
             x_proj_w, dt_proj_w, dt_proj_b, out_proj_w)
    weights = [_prep_weights(h, *[np.asarray(a, np.float32) for a in wargs])
               for h in range(2)]

    xtps = []
    for g in range(B):
        xtp = np.zeros((NU, 3, 128, L + 2), bf)
        for u in range(NU):
            xs = x[g, u * L:(u + 1) * L, :]        # (L, 384)
            xT = np.ascontiguousarray(xs.T)        # (384, L)
            xtp[u, :, :, 1:L + 1] = xT.reshape(3, 128, L).astype(bf)
        xtps.append(xtp)

    in_maps = []
    for core in range(NCORES):
        g, h = divmod(core, 2)
        m = dict(weights[h])
        m["xtp"] = xtps[g]
        in_maps.append(m)

    nc_prog = _get_nc(ab_same)
    r = run_bass_kernel_spmd(nc_prog, in_maps, list(range(NCORES)))
    res = r.results

    out = np.empty((B, SEQ, DIM), np.float32)
    bo = np.asarray(out_proj_b, np.float32)
    for g in range(B):
        for u in range(NU):
            part = (res[2 * g]["out"][u] + res[2 * g + 1]["out"][u] + bo)
            out[g, u * L:(u + 1) * L, :] = part
    return out
